# revision 41
# baseline (speedup 1.0000x reference)
"""Trainium2 Bass kernel for masked-dropout attention-score matmul.

Computes, for q/k/v [B,H,S,D] and an int32 0/1 keep-mask [B,H,S,S]:

    out = ((q @ k^T) * sqrt(D) * 2 * mask) @ v        (2 = 1/(1-p_drop))

Strategy (8 NeuronCores, SPMD, no collectives) — default variant "v6":
  - Shard the 32 (b,h) pairs 4-per-core; everything ships as fp16
    (total rel err ~4.6e-4 vs the 2e-2 budget).
  - Per pair and q-half of 1024 columns, accumulate O^T over 16 k-chunks:
    mm1 S^T-chunk = K_chunk @ Q^T-half into PSUM (f32), mask it into
    fp16 SBUF, mm2 O^T += V_chunk^T @ S'^T.  PSUM: 3x st[128,1024]
    double-bank tiles + 1 out accumulator = all 8 banks.
  - Masking is split across engines to keep everything under the PE's
    ~133 us/core floor: half the k-chunk units use a DVE-fused multiply
    (f32 PSUM x fp8 {0,1.0} mask -> f16, 1x mode); the other half go
    ACT-evict (PSUM->f16 SBUF) + DVE bitwise-AND with a u16 0xFFFF/0
    mask (2x mode: all-SBUF, 2-byte, packed).
  - mm2 is software-pipelined 4 k-chunks behind mm1 so the in-order PE
    stream never waits on the masking chain; masks are DMAed on the
    otherwise-idle GpSimd DGE queue, chunked so the first units land
    within ~2 us; the first pair's q/k/v loads are split for cold-start.
  - The scale (2*sqrt(D)) is folded into V on the host; Q^T/K^T/V and
    both mask streams are host-rearranged so all device DMAs are fully
    contiguous.
"""

import os
import sys

sys.path.insert(0, "/opt/trn_rl_repo")

import numpy as np

import concourse.bacc as bacc
import concourse.bass as bass
import concourse.mybir as mybir
import concourse.tile as tile
from concourse.bass_utils import run_bass_kernel_spmd

B, H, SQ, SK, D = 2, 16, 2048, 2048, 128
P_DROP = 0.5
SCALE = float(D) ** 0.5 / (1.0 - P_DROP)  # folded into V on the host
N_CORES = 8
PAIRS = B * H
PAIRS_PER_CORE = PAIRS // N_CORES

F32 = mybir.dt.float32
F32R = mybir.dt.float32r
FP8 = mybir.dt.float8e4
U8 = mybir.dt.uint8
U16 = mybir.dt.uint16
F16 = mybir.dt.float16
BF16 = mybir.dt.bfloat16

FP8_ONE = 0x38  # float8_e4m3 encoding of 1.0
FUSED_COLS = 512  # v4: q-columns masked via the DVE-fused fp8 path

# module-level handle for test.py to inspect timing after a traced run
LAST_RESULTS = None


def emit_body(nc, tc, ot, qt, kt, v, mt, n_pairs, sq, sk, d=D, qn=512, repeat=1,
              loop_n=1, mmdt=F32R):
    """Emit the per-core program.

    APs (all on this core's DRAM):
      qt [n_pairs, d,  sq]  f32  : Q^T per pair
      kt [n_pairs, d,  sk]  f32  : K^T per pair
      v  [n_pairs, d?, ...]      : V rearranged to [128, (sk//128)*d], f32,
                                   v[p][r][c*d+j] = V[c*128+r, j] * SCALE
      mt [n_pairs, sk, sq]  u8   : mask^T as fp8 bytes (0x00 / 0x38)
      ot [n_pairs, d,  sq]  f32  : O^T output
    """
    nkc = sk // 128
    nqc = sq // qn

    import contextlib

    with contextlib.ExitStack() as ctx:
        qt_pool = ctx.enter_context(tc.tile_pool(name="qt", bufs=2))
        kt_pool = ctx.enter_context(tc.tile_pool(name="kt", bufs=2))
        v_pool = ctx.enter_context(tc.tile_pool(name="v", bufs=2))
        m_pool = ctx.enter_context(tc.tile_pool(name="m", bufs=4))
        sp_pool = ctx.enter_context(tc.tile_pool(name="sp", bufs=6))
        o_pool = ctx.enter_context(tc.tile_pool(name="o", bufs=2))
        st_pool = ctx.enter_context(tc.tile_pool(name="st", bufs=4, space="PSUM"))
        ot_pool = ctx.enter_context(tc.tile_pool(name="otp", bufs=1, space="PSUM"))

        loop_cm = tc.For_i(0, loop_n, 1) if loop_n > 1 else contextlib.nullcontext()
        with loop_cm:
          for p in [pp for _ in range(repeat) for pp in range(n_pairs)]:
            qt_t = qt_pool.tile([128, sq], mmdt)
            nc.sync.dma_start(out=qt_t[:d], in_=qt[p])
            kt_t = kt_pool.tile([128, sk], mmdt)
            nc.sync.dma_start(out=kt_t[:d], in_=kt[p])
            v_t = v_pool.tile([128, nkc * d], mmdt)
            nc.sync.dma_start(out=v_t[:], in_=v[p])

            ot_ps = ot_pool.tile([128, sq], F32)

            for kc in range(nkc):
                m_t = m_pool.tile([128, sq], U8)
                nc.sync.dma_start(out=m_t[:], in_=mt[p, kc * 128 : (kc + 1) * 128, :])

                for qc in range(nqc):
                    st = st_pool.tile([128, qn], F32)
                    nc.tensor.matmul(
                        st[:],
                        kt_t[:d, kc * 128 : (kc + 1) * 128],
                        qt_t[:d, qc * qn : (qc + 1) * qn],
                        start=True,
                        stop=True,
                    )
                    sp = sp_pool.tile([128, qn], mmdt)
                    nc.vector.tensor_mul(
                        sp[:],
                        st[:],
                        m_t[:, qc * qn : (qc + 1) * qn].bitcast(FP8),
                    )
                    nc.tensor.matmul(
                        ot_ps[:d, qc * qn : (qc + 1) * qn],
                        v_t[:, kc * d : (kc + 1) * d],
                        sp[:],
                        start=(kc == 0),
                        stop=(kc == nkc - 1),
                    )

            o_t = o_pool.tile([128, sq], F32)
            nc.scalar.copy(o_t[:d], ot_ps[:d])
            nc.sync.dma_start(out=ot[p], in_=o_t[:d])


def emit_body_v2(
    nc, tc, ot, qt, kt, v, mt, n_pairs, sq, sk, d=D, qn=512, gn=1024, fused_mod=(1, 4),
    repeat=1, loop_n=1,
):
    """Balanced-engine variant.

    Masking is split across three engines per [128, gn] score group:
      - fused path (idx % fused_mod[1] < fused_mod[0]): DVE multiplies
        PSUM f32 scores by the fp8 mask directly -> bf16 SBUF.
      - split path: ACT evicts PSUM f32 -> bf16 SBUF, GpSimd converts the
        fp8 mask -> bf16, DVE multiplies bf16 x bf16 in its 2x mode.
    Second matmul runs with bf16 moving operand at N=gn; V ships as bf16.
    """
    nkc = sk // 128
    ngc = sq // gn

    import contextlib

    with contextlib.ExitStack() as ctx:
        qt_pool = ctx.enter_context(tc.tile_pool(name="qt", bufs=2))
        kt_pool = ctx.enter_context(tc.tile_pool(name="kt", bufs=2))
        v_pool = ctx.enter_context(tc.tile_pool(name="v", bufs=2))
        m_pool = ctx.enter_context(tc.tile_pool(name="m", bufs=4))
        sp_pool = ctx.enter_context(tc.tile_pool(name="sp", bufs=6))
        se_pool = ctx.enter_context(tc.tile_pool(name="se", bufs=4))
        mb_pool = ctx.enter_context(tc.tile_pool(name="mb", bufs=4))
        o_pool = ctx.enter_context(tc.tile_pool(name="o", bufs=2))
        st_pool = ctx.enter_context(tc.tile_pool(name="st", bufs=2, space="PSUM"))
        ot_pool = ctx.enter_context(tc.tile_pool(name="otp", bufs=1, space="PSUM"))

        unit = 0
        loop_cm = tc.For_i(0, loop_n, 1) if loop_n > 1 else contextlib.nullcontext()
        with loop_cm:
          for p in [pp for _ in range(repeat) for pp in range(n_pairs)]:
            qt_t = qt_pool.tile([128, sq], F32R)
            nc.sync.dma_start(out=qt_t[:d], in_=qt[p])
            kt_t = kt_pool.tile([128, sk], F32R)
            nc.sync.dma_start(out=kt_t[:d], in_=kt[p])
            v_t = v_pool.tile([128, nkc * d], BF16)
            nc.sync.dma_start(out=v_t[:], in_=v[p])

            ot_ps = ot_pool.tile([128, sq], F32)

            for kc in range(nkc):
                m_t = m_pool.tile([128, sq], U8)
                nc.sync.dma_start(out=m_t[:], in_=mt[p, kc * 128 : (kc + 1) * 128, :])

                for g in range(ngc):
                    st = st_pool.tile([128, gn], F32)
                    for j in range(gn // qn):
                        c0 = g * gn + j * qn
                        nc.tensor.matmul(
                            st[:, j * qn : (j + 1) * qn],
                            kt_t[:d, kc * 128 : (kc + 1) * 128],
                            qt_t[:d, c0 : c0 + qn],
                            start=True,
                            stop=True,
                        )
                    m_sl = m_t[:, g * gn : (g + 1) * gn].bitcast(FP8)
                    sp = sp_pool.tile([128, gn], BF16)
                    if unit % fused_mod[1] < fused_mod[0]:
                        nc.vector.tensor_mul(sp[:], st[:], m_sl)
                    else:
                        se = se_pool.tile([128, gn], BF16)
                        nc.scalar.copy(se[:], st[:])
                        mb = mb_pool.tile([128, gn], BF16)
                        nc.gpsimd.tensor_copy(mb[:], m_sl)
                        nc.vector.tensor_mul(sp[:], se[:], mb[:])
                    unit += 1
                    for j in range(gn // qn):
                        c0 = g * gn + j * qn
                        nc.tensor.matmul(
                            ot_ps[:d, c0 : c0 + qn],
                            v_t[:, kc * d : (kc + 1) * d],
                            sp[:, j * qn : (j + 1) * qn],
                            start=(kc == 0),
                            stop=(kc == nkc - 1),
                        )

            o_t = o_pool.tile([128, sq], F32)
            nc.scalar.copy(o_t[:d], ot_ps[:d])
            nc.sync.dma_start(out=ot[p], in_=o_t[:d])


def emit_body_v4(nc, tc, ot, qt, kt, v, mt8, mt16, n_pairs, sq, sk, d=D, repeat=1,
                 loop_n=1, fused_cols=512):
    """DVE-decongested variant.

    Per (pair, q-half of 1024): accumulate O^T over 16 k-chunks.
      - mm1: S^T chunk = K_chunk @ Q^T-half -> PSUM st [128, 1024] f32
      - masking splits by q-column range:
          * cols [0, fused_cols) of the full q-range: DVE fused
            tensor_mul(f32 PSUM x fp8 mask -> f16 SBUF) at 1x
          * the rest: ACT evicts PSUM -> f16 SBUF, DVE bitwise-ANDs with a
            u16 0xFFFF/0 mask at 2x (all-SBUF, 2-byte, packed)
      - mm2: O^T += V_chunk^T @ sp  (f16 moving operand)
    PSUM: ot 2 banks x2 bufs + st 2 banks x2 bufs = 8 banks.
    """
    import contextlib

    nkc = sk // 128
    hn = 1024
    nh = sq // hn

    with contextlib.ExitStack() as ctx:
        qt_pool = ctx.enter_context(tc.tile_pool(name="qt", bufs=2))
        kt_pool = ctx.enter_context(tc.tile_pool(name="kt", bufs=2))
        v_pool = ctx.enter_context(tc.tile_pool(name="v", bufs=2))
        m8_pool = ctx.enter_context(tc.tile_pool(name="m8", bufs=2))
        m16_pool = ctx.enter_context(tc.tile_pool(name="m16", bufs=2))
        se_pool = ctx.enter_context(tc.tile_pool(name="se", bufs=4))
        sp_pool = ctx.enter_context(tc.tile_pool(name="sp", bufs=4))
        o_pool = ctx.enter_context(tc.tile_pool(name="o", bufs=2))
        st_pool = ctx.enter_context(tc.tile_pool(name="st", bufs=2, space="PSUM"))
        ot_pool = ctx.enter_context(tc.tile_pool(name="otp", bufs=2, space="PSUM"))

        bw_tot = sq - fused_cols  # band-masked columns per k-chunk

        loop_cm = tc.For_i(0, loop_n, 1) if loop_n > 1 else contextlib.nullcontext()
        with loop_cm:
          for p in [pp for _ in range(repeat) for pp in range(n_pairs)]:
            qt_t = qt_pool.tile([128, sq], F16)
            nc.sync.dma_start(out=qt_t[:d], in_=qt[p])
            kt_t = kt_pool.tile([128, sk], F16)
            nc.sync.dma_start(out=kt_t[:d], in_=kt[p])
            v_t = v_pool.tile([128, nkc * d], F16)
            nc.sync.dma_start(out=v_t[:], in_=v[p])
            # whole-pair masks, chunk-major: one contiguous DMA per stream
            m8_t = m8_pool.tile([128, nkc * fused_cols], U8)
            nc.sync.dma_start(out=m8_t[:], in_=mt8[p])
            m16_t = m16_pool.tile([128, nkc * bw_tot], U16)
            nc.sync.dma_start(out=m16_t[:], in_=mt16[p])

            for h in range(nh):
                c0 = h * hn  # global q-column base of this half
                # fused (fp8-mask) columns within this half
                f_lo = min(max(fused_cols - c0, 0), hn)
                ot_ps = ot_pool.tile([128, hn], F32)

                for kc in range(nkc):
                    st = st_pool.tile([128, hn], F32)
                    for j in range(hn // 512):
                        nc.tensor.matmul(
                            st[:, j * 512 : (j + 1) * 512],
                            kt_t[:d, kc * 128 : (kc + 1) * 128],
                            qt_t[:d, c0 + j * 512 : c0 + (j + 1) * 512],
                            start=True,
                            stop=True,
                        )
                    sp = sp_pool.tile([128, hn], F16)
                    if f_lo > 0:
                        nc.vector.tensor_mul(
                            sp[:, :f_lo],
                            st[:, :f_lo],
                            m8_t[
                                :, kc * fused_cols + c0 : kc * fused_cols + c0 + f_lo
                            ].bitcast(FP8),
                        )
                    if f_lo < hn:
                        bw = hn - f_lo
                        b0 = kc * bw_tot + c0 + f_lo - fused_cols
                        se = se_pool.tile([128, bw], F16)
                        nc.scalar.copy(se[:], st[:, f_lo:hn])
                        nc.vector.tensor_tensor(
                            sp[:, f_lo:hn].bitcast(U16),
                            se[:].bitcast(U16),
                            m16_t[:, b0 : b0 + bw],
                            mybir.AluOpType.bitwise_and,
                        )
                    for j in range(hn // 512):
                        nc.tensor.matmul(
                            ot_ps[:d, j * 512 : (j + 1) * 512],
                            v_t[:, kc * d : (kc + 1) * d],
                            sp[:, j * 512 : (j + 1) * 512],
                            start=(kc == 0),
                            stop=(kc == nkc - 1),
                        )

                o_t = o_pool.tile([128, hn], F16)
                nc.scalar.copy(o_t[:d], ot_ps[:d])
                nc.sync.dma_start(out=ot[p, :, c0 : c0 + hn], in_=o_t[:d])


def emit_body_v5(nc, tc, ot, qt, kt, v, mt8, mt16, n_pairs, sq, sk, d=D, repeat=1,
                 loop_n=1):
    """Deep-pipelined variant: 512-col PSUM subtiles, 6-deep st pipeline,
    fused-fp8 subtile alternates q-half by kc parity to balance ACT vs DVE.

    Per (pair, q-half of 1024), accumulate O^T over 16 k-chunks; per k-chunk
    two 512-col subtiles. One subtile per (kc) [in the kc-parity-matched
    half] is masked by the DVE-fused fp8 path; the other three go
    ACT-evict + DVE-band (2x).

    mt8 [p, 128, nkc*512] u8 chunk-major: fused cols of each kc
      (kc even: q[0:512], kc odd: q[1024:1536])
    mt16 [p, 128, nkc*1536] u16 chunk-major, band cols compacted per kc
      (kc even: q[512:2048]; kc odd: q[0:1024] ++ q[1536:2048])
    """
    import contextlib

    nkc = sk // 128
    hn = 1024
    nh = sq // hn

    with contextlib.ExitStack() as ctx:
        qt_pool = ctx.enter_context(tc.tile_pool(name="qt", bufs=2))
        kt_pool = ctx.enter_context(tc.tile_pool(name="kt", bufs=2))
        v_pool = ctx.enter_context(tc.tile_pool(name="v", bufs=2))
        m8_pool = ctx.enter_context(tc.tile_pool(name="m8", bufs=2))
        m16_pool = ctx.enter_context(tc.tile_pool(name="m16", bufs=2))
        se_pool = ctx.enter_context(tc.tile_pool(name="se", bufs=6))
        sp_pool = ctx.enter_context(tc.tile_pool(name="sp", bufs=8))
        o_pool = ctx.enter_context(tc.tile_pool(name="o", bufs=2))
        st_pool = ctx.enter_context(tc.tile_pool(name="st", bufs=6, space="PSUM"))
        ot_pool = ctx.enter_context(tc.tile_pool(name="otp", bufs=1, space="PSUM"))

        loop_cm = tc.For_i(0, loop_n, 1) if loop_n > 1 else contextlib.nullcontext()
        with loop_cm:
          first_p = True
          for p in [pp for _ in range(repeat) for pp in range(n_pairs)]:
            qt_t = qt_pool.tile([128, sq], F16)
            kt_t = kt_pool.tile([128, sk], F16)
            v_t = v_pool.tile([128, nkc * d], F16)
            if first_p:
                nc.sync.dma_start(out=kt_t[:d, :256], in_=kt[p, :, :256])
                nc.sync.dma_start(out=qt_t[:d, :1024], in_=qt[p, :, :1024])
                nc.sync.dma_start(out=v_t[:, : 2 * d], in_=v[p, :, : 2 * d])
                nc.sync.dma_start(out=kt_t[:d, 256:], in_=kt[p, :, 256:])
                nc.sync.dma_start(out=qt_t[:d, 1024:], in_=qt[p, :, 1024:])
                nc.sync.dma_start(out=v_t[:, 2 * d :], in_=v[p, :, 2 * d :])
                first_p = False
            else:
                nc.sync.dma_start(out=qt_t[:d], in_=qt[p])
                nc.sync.dma_start(out=kt_t[:d], in_=kt[p])
                nc.sync.dma_start(out=v_t[:], in_=v[p])
            # masks on the GpSimd queue, kc-chunked for fast cold-start
            m8_t = m8_pool.tile([128, nkc * 512], U8)
            m16_t = m16_pool.tile([128, nkc * 1536], U16)
            nc.gpsimd.dma_start(out=m8_t[:, : 2 * 512], in_=mt8[p, :, : 2 * 512])
            nc.gpsimd.dma_start(out=m16_t[:, : 2 * 1536], in_=mt16[p, :, : 2 * 1536])
            nc.gpsimd.dma_start(out=m8_t[:, 2 * 512 :], in_=mt8[p, :, 2 * 512 :])
            nc.gpsimd.dma_start(
                out=m16_t[:, 2 * 1536 : 6 * 1536], in_=mt16[p, :, 2 * 1536 : 6 * 1536]
            )
            nc.gpsimd.dma_start(out=m16_t[:, 6 * 1536 :], in_=mt16[p, :, 6 * 1536 :])

            for h in range(nh):
                c0 = h * hn
                ot_ps = ot_pool.tile([128, hn], F32)

                for kc in range(nkc):
                    for j in range(2):
                        q0 = c0 + j * 512  # global q-col base of subtile
                        st = st_pool.tile([128, 512], F32)
                        nc.tensor.matmul(
                            st[:],
                            kt_t[:d, kc * 128 : (kc + 1) * 128],
                            qt_t[:d, q0 : q0 + 512],
                            start=True,
                            stop=True,
                        )
                        sp = sp_pool.tile([128, 512], F16)
                        fused = (kc % 2 == h) and j == 0
                        if fused:
                            nc.vector.tensor_mul(
                                sp[:],
                                st[:],
                                m8_t[:, kc * 512 : (kc + 1) * 512].bitcast(FP8),
                            )
                        else:
                            # compacted band-column index of this subtile
                            if kc % 2 == 0:
                                b0 = q0 - 512  # band cols are q[512:2048]
                            else:
                                b0 = q0 if q0 < 1024 else q0 - 512
                            se = se_pool.tile([128, 512], F16)
                            nc.scalar.copy(se[:], st[:])
                            nc.vector.tensor_tensor(
                                sp[:].bitcast(U16),
                                se[:].bitcast(U16),
                                m16_t[:, kc * 1536 + b0 : kc * 1536 + b0 + 512],
                                mybir.AluOpType.bitwise_and,
                            )
                        nc.tensor.matmul(
                            ot_ps[:d, j * 512 : (j + 1) * 512],
                            v_t[:, kc * d : (kc + 1) * d],
                            sp[:],
                            start=(kc == 0),
                            stop=(kc == nkc - 1),
                        )

                o_t = o_pool.tile([128, hn], F16)
                nc.scalar.copy(o_t[:d], ot_ps[:d])
                nc.sync.dma_start(out=ot[p, :, c0 : c0 + hn], in_=o_t[:d])


_FUSED_SETS = {
    "3": (2, 7, 12),
    "4": (1, 5, 9, 13),
    "5": (1, 4, 7, 10, 13),
    "6": (1, 3, 6, 9, 11, 14),
    "8": (1, 3, 5, 7, 9, 11, 13, 15),
}


def _fused_kc():
    # kc indices (per half) whose whole kc-unit is DVE-fused
    return _FUSED_SETS[os.environ.get("V6_FUSED", "8")]


def _v6_slots(nkc, nh):
    """Processing-order (h, kc) slots -> ('f'|'b', running index within type)."""
    fused_kc = _fused_kc()
    slots = {}
    i8 = i16 = 0
    for h in range(nh):
        for kc in range(nkc):
            if kc in fused_kc:
                slots[(h, kc)] = ("f", i8)
                i8 += 1
            else:
                slots[(h, kc)] = ("b", i16)
                i16 += 1
    return slots, i8, i16


def emit_body_v6(nc, tc, ot, qt, kt, v, mt8, mt16, n_pairs, sq, sk, d=D, repeat=1,
                 loop_n=1):
    """Coarse kc-unit variant: whole [128, 1024] kc-units are either DVE-fused
    (f32 PSUM x fp8 -> f16, 1x) or ACT-evict + DVE-band (u16 AND, 2x).

    st [128,1024] PSUM x3 bufs + ot [128,1024] x1 = 8 banks.
    mt8 [p, 128, n_f*1024] u8, mt16 [p, 128, n_b*1024] u16, slot-major in
    processing order (h outer, kc inner).
    """
    import contextlib

    nkc = sk // 128
    hn = 1024
    nh = sq // hn
    slots, n_f, n_b = _v6_slots(nkc, nh)
    # slot counts in h0, for the split mask prefetch
    f_h0 = sum(1 for (h, kc), (t, i) in slots.items() if h == 0 and t == "f")
    b_h0 = sum(1 for (h, kc), (t, i) in slots.items() if h == 0 and t == "b")

    with contextlib.ExitStack() as ctx:
        qt_pool = ctx.enter_context(tc.tile_pool(name="qt", bufs=2))
        kt_pool = ctx.enter_context(tc.tile_pool(name="kt", bufs=2))
        v_pool = ctx.enter_context(tc.tile_pool(name="v", bufs=2))
        m8_pool = ctx.enter_context(tc.tile_pool(name="m8", bufs=2))
        m16_pool = ctx.enter_context(tc.tile_pool(name="m16", bufs=2))
        st_bufs = int(os.environ.get("V6_ST_BUFS", "3"))
        ot_bufs = int(os.environ.get("V6_OT_BUFS", "1"))
        split_band = os.environ.get("V6_SPLIT_BAND", "0") == "1"
        lag = int(os.environ.get("V6_LAG", "4"))
        se_pool = ctx.enter_context(tc.tile_pool(name="se", bufs=5))
        sp_pool = ctx.enter_context(tc.tile_pool(name="sp", bufs=int(os.environ.get("V6_SP_BUFS", "7"))))
        o_pool = ctx.enter_context(tc.tile_pool(name="o", bufs=2))
        st_pool = ctx.enter_context(tc.tile_pool(name="st", bufs=st_bufs, space="PSUM"))
        ot_pool = ctx.enter_context(tc.tile_pool(name="otp", bufs=ot_bufs, space="PSUM"))

        loop_cm = tc.For_i(0, loop_n, 1) if loop_n > 1 else contextlib.nullcontext()
        with loop_cm:
          first_p = True
          for p in [pp for _ in range(repeat) for pp in range(n_pairs)]:
            qt_t = qt_pool.tile([128, sq], F16)
            kt_t = kt_pool.tile([128, sk], F16)
            v_t = v_pool.tile([128, nkc * d], F16)
            if first_p:
                # split first-pair inputs so kc0 compute starts ASAP
                nc.sync.dma_start(out=kt_t[:d, :256], in_=kt[p, :, :256])
                nc.sync.dma_start(out=qt_t[:d, :hn], in_=qt[p, :, :hn])
                nc.sync.dma_start(out=v_t[:, : 2 * d], in_=v[p, :, : 2 * d])
                nc.sync.dma_start(out=kt_t[:d, 256:], in_=kt[p, :, 256:])
                nc.sync.dma_start(out=qt_t[:d, hn:], in_=qt[p, :, hn:])
                nc.sync.dma_start(out=v_t[:, 2 * d :], in_=v[p, :, 2 * d :])
                first_p = False
            else:
                nc.sync.dma_start(out=qt_t[:d], in_=qt[p])
                nc.sync.dma_start(out=kt_t[:d], in_=kt[p])
                nc.sync.dma_start(out=v_t[:], in_=v[p])
            # masks on the GpSimd DMA queue, chunked so the first kc-units land
            # ASAP; flows in parallel with the sync queue
            m8_t = m8_pool.tile([128, n_f * hn], U8)
            m16_t = m16_pool.tile([128, n_b * hn], U16)
            for lo, hi in ((0, 2), (2, 5), (5, b_h0), (b_h0, n_b)):
                if lo < hi:
                    nc.gpsimd.dma_start(
                        out=m16_t[:, lo * hn : hi * hn],
                        in_=mt16[p, :, lo * hn : hi * hn],
                    )
                if lo == 0:
                    nc.gpsimd.dma_start(
                        out=m8_t[:, : f_h0 * hn], in_=mt8[p, :, : f_h0 * hn]
                    )
                elif lo == b_h0:
                    nc.gpsimd.dma_start(
                        out=m8_t[:, f_h0 * hn :], in_=mt8[p, :, f_h0 * hn :]
                    )

            for h in range(nh):
                c0 = h * hn
                ot_ps = ot_pool.tile([128, hn], F32)

                # software-pipelined by `lag` kc stages: PE emits mm1(kc)
                # before mm2(kc-lag) so the in-order PE stream gives the mask
                # chain lag*1µs to produce sp (sp tiles buffer it in SBUF)
                sps = {}
                for kc in range(nkc + lag):
                    if kc < nkc:
                        st = st_pool.tile([128, hn], F32)
                        for j in range(2):
                            nc.tensor.matmul(
                                st[:, j * 512 : (j + 1) * 512],
                                kt_t[:d, kc * 128 : (kc + 1) * 128],
                                qt_t[:d, c0 + j * 512 : c0 + (j + 1) * 512],
                                start=True,
                                stop=True,
                            )
                        sp = sp_pool.tile([128, hn], F16, name="sp")
                        sps[kc] = sp
                        typ, idx = slots[(h, kc)]
                        if typ == "f":
                            nc.vector.tensor_mul(
                                sp[:],
                                st[:],
                                m8_t[:, idx * hn : (idx + 1) * hn].bitcast(FP8),
                            )
                        elif split_band:
                            # per-512 subtile evict+band: shorter chain latency
                            se = se_pool.tile([128, hn], F16)
                            for j in range(2):
                                sl = slice(j * 512, (j + 1) * 512)
                                nc.scalar.copy(se[:, sl], st[:, sl])
                                nc.vector.tensor_tensor(
                                    sp[:, sl].bitcast(U16),
                                    se[:, sl].bitcast(U16),
                                    m16_t[:, idx * hn + j * 512 : idx * hn + (j + 1) * 512],
                                    mybir.AluOpType.bitwise_and,
                                )
                        else:
                            se = se_pool.tile([128, hn], F16)
                            nc.scalar.copy(se[:], st[:])
                            nc.vector.tensor_tensor(
                                sp[:].bitcast(U16),
                                se[:].bitcast(U16),
                                m16_t[:, idx * hn : (idx + 1) * hn],
                                mybir.AluOpType.bitwise_and,
                            )
                    if kc >= lag:
                        kd = kc - lag
                        sp = sps.pop(kd)
                        for j in range(2):
                            nc.tensor.matmul(
                                ot_ps[:d, j * 512 : (j + 1) * 512],
                                v_t[:, kd * d : (kd + 1) * d],
                                sp[:, j * 512 : (j + 1) * 512],
                                start=(kd == 0),
                                stop=(kd == nkc - 1),
                            )

                o_t = o_pool.tile([128, hn], F16)
                if p == n_pairs - 1 and h == nh - 1:
                    # split the final copy so the drain tail overlaps
                    for j in range(2):
                        nc.scalar.copy(
                            o_t[:d, j * 512 : (j + 1) * 512],
                            ot_ps[:d, j * 512 : (j + 1) * 512],
                        )
                        nc.sync.dma_start(
                            out=ot[p, :, c0 + j * 512 : c0 + (j + 1) * 512],
                            in_=o_t[:d, j * 512 : (j + 1) * 512],
                        )
                else:
                    nc.scalar.copy(o_t[:d], ot_ps[:d])
                    nc.sync.dma_start(out=ot[p, :, c0 : c0 + hn], in_=o_t[:d])


def build_nc(n_pairs=PAIRS_PER_CORE, sq=SQ, sk=SK, d=D, qn=512, variant="v1", repeat=1,
             loop_n=1):
    nc = bacc.Bacc("TRN2", target_bir_lowering=False, debug=False)
    if variant == "v6":
        nkc = sk // 128
        nh = sq // 1024
        _, n_f, n_b = _v6_slots(nkc, nh)
        qt = nc.declare_dram_parameter("qt", [n_pairs, d, sq], F16, isOutput=False)
        kt = nc.declare_dram_parameter("kt", [n_pairs, d, sk], F16, isOutput=False)
        v = nc.declare_dram_parameter(
            "v", [n_pairs, 128, nkc * d], F16, isOutput=False
        )
        mt8 = nc.declare_dram_parameter(
            "mt8", [n_pairs, 128, n_f * 1024], U8, isOutput=False
        )
        mt16 = nc.declare_dram_parameter(
            "mt16", [n_pairs, 128, n_b * 1024], U16, isOutput=False
        )
        ot = nc.declare_dram_parameter("ot", [n_pairs, d, sq], F16, isOutput=True)
        with tile.TileContext(nc) as tc:
            emit_body_v6(nc, tc, ot, qt, kt, v, mt8, mt16, n_pairs, sq, sk, d,
                         repeat=repeat, loop_n=loop_n)
        nc.compile()
        return nc
    if variant == "v5":
        nkc = sk // 128
        qt = nc.declare_dram_parameter("qt", [n_pairs, d, sq], F16, isOutput=False)
        kt = nc.declare_dram_parameter("kt", [n_pairs, d, sk], F16, isOutput=False)
        v = nc.declare_dram_parameter(
            "v", [n_pairs, 128, nkc * d], F16, isOutput=False
        )
        mt8 = nc.declare_dram_parameter(
            "mt8", [n_pairs, 128, nkc * 512], U8, isOutput=False
        )
        mt16 = nc.declare_dram_parameter(
            "mt16", [n_pairs, 128, nkc * 1536], U16, isOutput=False
        )
        ot = nc.declare_dram_parameter("ot", [n_pairs, d, sq], F16, isOutput=True)
        with tile.TileContext(nc) as tc:
            emit_body_v5(nc, tc, ot, qt, kt, v, mt8, mt16, n_pairs, sq, sk, d,
                         repeat=repeat, loop_n=loop_n)
        nc.compile()
        return nc
    if variant == "v4":
        fused_cols = FUSED_COLS
        qt = nc.declare_dram_parameter("qt", [n_pairs, d, sq], F16, isOutput=False)
        kt = nc.declare_dram_parameter("kt", [n_pairs, d, sk], F16, isOutput=False)
        v = nc.declare_dram_parameter(
            "v", [n_pairs, 128, (sk // 128) * d], F16, isOutput=False
        )
        nkc = sk // 128
        mt8 = nc.declare_dram_parameter(
            "mt8", [n_pairs, 128, nkc * fused_cols], U8, isOutput=False
        )
        mt16 = nc.declare_dram_parameter(
            "mt16", [n_pairs, 128, nkc * (sq - fused_cols)], U16, isOutput=False
        )
        ot = nc.declare_dram_parameter("ot", [n_pairs, d, sq], F16, isOutput=True)
        with tile.TileContext(nc) as tc:
            emit_body_v4(nc, tc, ot, qt, kt, v, mt8, mt16, n_pairs, sq, sk, d,
                         repeat=repeat, loop_n=loop_n, fused_cols=fused_cols)
        nc.compile()
        return nc
    mmdt = F32R if variant == "v1" else BF16
    vdt = mmdt
    qt = nc.declare_dram_parameter("qt", [n_pairs, d, sq], mmdt, isOutput=False)
    kt = nc.declare_dram_parameter("kt", [n_pairs, d, sk], mmdt, isOutput=False)
    v = nc.declare_dram_parameter("v", [n_pairs, 128, (sk // 128) * d], vdt, isOutput=False)
    mt = nc.declare_dram_parameter("mt", [n_pairs, sk, sq], U8, isOutput=False)
    ot = nc.declare_dram_parameter("ot", [n_pairs, d, sq], F32, isOutput=True)
    with tile.TileContext(nc) as tc:
        if variant in ("v1", "v3"):
            emit_body(nc, tc, ot, qt, kt, v, mt, n_pairs, sq, sk, d, qn, repeat=repeat,
                      loop_n=loop_n, mmdt=mmdt)
        else:
            emit_body_v2(nc, tc, ot, qt, kt, v, mt, n_pairs, sq, sk, d, qn,
                         repeat=repeat, loop_n=loop_n)
    nc.compile()
    return nc


def _prep_inputs(query, key, value, dropout_mask, variant="v1"):
    """Host-side marshaling into per-core input maps."""
    import ml_dtypes

    q = np.asarray(query, dtype=np.float32).reshape(PAIRS, SQ, D)
    k = np.asarray(key, dtype=np.float32).reshape(PAIRS, SK, D)
    vv = np.asarray(value, dtype=np.float32).reshape(PAIRS, SK, D)
    m = np.asarray(dropout_mask).reshape(PAIRS, SQ, SK)

    qt = np.ascontiguousarray(q.transpose(0, 2, 1))  # [PAIRS, D, SQ]
    kt = np.ascontiguousarray(k.transpose(0, 2, 1))  # [PAIRS, D, SK]
    # V * SCALE rearranged: vr[p][r][c*D+j] = V[c*128+r, j] * SCALE
    vr = (vv * np.float32(SCALE)).reshape(PAIRS, SK // 128, 128, D)
    vr = np.ascontiguousarray(vr.transpose(0, 2, 1, 3)).reshape(PAIRS, 128, (SK // 128) * D)

    if variant == "v6":
        qt = qt.astype(np.float16)
        kt = kt.astype(np.float16)
        vr = vr.astype(np.float16)
        nkc = SK // 128
        nh = SQ // 1024
        slots, n_f, n_b = _v6_slots(nkc, nh)
        mcm = (m != 0).transpose(0, 2, 1).reshape(PAIRS, nkc, 128, SQ)
        mcm = np.ascontiguousarray(mcm.transpose(0, 2, 1, 3))  # [P,128,nkc,SQ]
        mt8 = np.zeros((PAIRS, 128, n_f * 1024), np.uint8)
        mt16 = np.zeros((PAIRS, 128, n_b * 1024), np.uint16)
        for (h, kc), (typ, idx) in slots.items():
            blk = mcm[:, :, kc, h * 1024 : (h + 1) * 1024]
            if typ == "f":
                mt8[:, :, idx * 1024 : (idx + 1) * 1024] = blk.astype(
                    np.uint8
                ) * np.uint8(FP8_ONE)
            else:
                mt16[:, :, idx * 1024 : (idx + 1) * 1024] = blk.astype(
                    np.uint16
                ) * np.uint16(0xFFFF)
        in_maps = []
        for c in range(N_CORES):
            s = slice(c * PAIRS_PER_CORE, (c + 1) * PAIRS_PER_CORE)
            in_maps.append(
                {
                    "qt": qt[s],
                    "kt": kt[s],
                    "v": vr[s],
                    "mt8": np.ascontiguousarray(mt8[s]),
                    "mt16": np.ascontiguousarray(mt16[s]),
                }
            )
        return in_maps

    if variant == "v5":
        qt = qt.astype(np.float16)
        kt = kt.astype(np.float16)
        vr = vr.astype(np.float16)
        nkc = SK // 128
        mcm = (m != 0).transpose(0, 2, 1).reshape(PAIRS, nkc, 128, SQ)
        mcm = np.ascontiguousarray(mcm.transpose(0, 2, 1, 3))  # [P,128,nkc,SQ]
        fused = np.empty((PAIRS, 128, nkc, 512), np.bool_)
        band = np.empty((PAIRS, 128, nkc, 1536), np.bool_)
        ev, od = slice(0, None, 2), slice(1, None, 2)
        fused[:, :, ev] = mcm[:, :, ev, 0:512]
        fused[:, :, od] = mcm[:, :, od, 1024:1536]
        band[:, :, ev] = mcm[:, :, ev, 512:2048]
        band[:, :, od, :1024] = mcm[:, :, od, 0:1024]
        band[:, :, od, 1024:] = mcm[:, :, od, 1536:2048]
        mt8 = fused.astype(np.uint8).reshape(PAIRS, 128, -1) * np.uint8(FP8_ONE)
        mt16 = band.astype(np.uint16).reshape(PAIRS, 128, -1) * np.uint16(0xFFFF)
        in_maps = []
        for c in range(N_CORES):
            s = slice(c * PAIRS_PER_CORE, (c + 1) * PAIRS_PER_CORE)
            in_maps.append(
                {
                    "qt": qt[s],
                    "kt": kt[s],
                    "v": vr[s],
                    "mt8": np.ascontiguousarray(mt8[s]),
                    "mt16": np.ascontiguousarray(mt16[s]),
                }
            )
        return in_maps

    if variant == "v4":
        qt = qt.astype(np.float16)
        kt = kt.astype(np.float16)
        vr = vr.astype(np.float16)
        # chunk-major mask^T: mcm[p, r, kc, c] = mask[p, q=c, k=kc*128+r]
        mcm = (m != 0).transpose(0, 2, 1).reshape(PAIRS, SK // 128, 128, SQ)
        mcm = mcm.transpose(0, 2, 1, 3)  # [PAIRS, 128, nkc, SQ]
        mt8 = np.ascontiguousarray(
            mcm[:, :, :, :FUSED_COLS]
        ).astype(np.uint8).reshape(PAIRS, 128, -1) * np.uint8(FP8_ONE)
        mt16 = np.ascontiguousarray(
            mcm[:, :, :, FUSED_COLS:]
        ).astype(np.uint16).reshape(PAIRS, 128, -1) * np.uint16(0xFFFF)
        in_maps = []
        for c in range(N_CORES):
            s = slice(c * PAIRS_PER_CORE, (c + 1) * PAIRS_PER_CORE)
            in_maps.append(
                {
                    "qt": qt[s],
                    "kt": kt[s],
                    "v": vr[s],
                    "mt8": np.ascontiguousarray(mt8[s]),
                    "mt16": np.ascontiguousarray(mt16[s]),
                }
            )
        return in_maps

    if variant != "v1":
        vr = vr.astype(ml_dtypes.bfloat16)
        qt = qt.astype(ml_dtypes.bfloat16)
        kt = kt.astype(ml_dtypes.bfloat16)
    # mask^T as fp8 bytes
    mb = (m != 0).astype(np.uint8) * np.uint8(FP8_ONE)  # [PAIRS, SQ, SK] u8
    mbt = np.ascontiguousarray(mb.transpose(0, 2, 1))  # [PAIRS, SK, SQ]

    in_maps = []
    for c in range(N_CORES):
        s = slice(c * PAIRS_PER_CORE, (c + 1) * PAIRS_PER_CORE)
        in_maps.append(
            {
                "qt": qt[s],
                "kt": kt[s],
                "v": vr[s],
                "mt": mbt[s],
            }
        )
    return in_maps


def kernel(query, key, value, dropout_mask):
    global LAST_RESULTS
    variant = os.environ.get("KERNEL_VARIANT", "v6")
    in_maps = _prep_inputs(query, key, value, dropout_mask, variant)
    nc = build_nc(variant=variant)
    res = run_bass_kernel_spmd(nc, in_maps, list(range(N_CORES)), trace=False)
    LAST_RESULTS = res
    outs = np.concatenate([r["ot"] for r in res.results], axis=0)  # [PAIRS, D, SQ]
    out = outs.astype(np.float32).transpose(0, 2, 1).reshape(B, H, SQ, D)
    return np.ascontiguousarray(out)

